# revision 8
# baseline (speedup 1.0000x reference)
"""Trainium2 Bass kernel: fused concat-linear attention map + softmax.

reference:  scores[b,h,n] = key[b,n,:]@Wk[h,:] + query[b,0,:]@Wq[h,:] + bias[h]
            attn = softmax over n              (B=16, N=20000, D=256, H=8)

v4 structure (per core = 2 batches, each 20000 rows):
  * tiny operands (bf16 identity, zero-padded WkT stationaries, q@Wq+b
    biases, fold32 matrix) are precomputed on the host and DMA'd in.
  * p-MAJOR cast-DMA key loads (SWDGE, f32->bf16 during the HBM read):
    "(p s) d" puts 16 consecutive rows on one partition -> 16KB
    contiguous HBM reads per partition (vs 1KB in n-interleaved order),
    measurably faster wire (~6-8us/core).  The resulting within-set
    n-permutation (n = 16p + 4g + j) is undone on the HOST during the
    gather/unshard step (pure reshape/transpose, no math).
  * every set-load is split into 2 half-DMAs (first set: 4 quarters)
    so consumers wake on partial data; shortens ramp and tail.
  * 128x128 key-tile transposes as REGULAR bf16 matmuls vs bf16 identity
    (pipelined LDW+MM, FWL; avoids transpose-mode stalls).
  * PSUM->SBUF copies (f32 -> round bf16) split DVE/ACT by ratio.
  * score matmuls use ZERO-PADDED stationaries wk32[dhalf][g] ([128,32],
    cols 8g..8g+8 = WkT half): four 512-col sub-chunks accumulate into
    ONE [32,512] PSUM bank at partition groups g=0..3.  One exp covers
    2048 rows -> 4x fewer ACT exp cycles.
  * totals: ONE fold32 [32,32] matmul replaces the foldA/foldB round
    trip; reciprocal reads the PSUM result directly.
  * final scale muls round-robin DVE/ACT/Pool; store DMAs round-robin
    sync(/tensor/gpsimd on the last batch) so the tail is not
    issue-serialized on one engine.
"""

import sys

import numpy as np

for _p in ("/opt/trn_rl_repo",):
    if _p not in sys.path:
        sys.path.append(_p)

from contextlib import ExitStack

import ml_dtypes
import concourse.bass as bass
import concourse.bacc as bacc
import concourse.tile as tile
from concourse import mybir

B, N, D, H = 16, 20000, 256, 8
NCORES = 8
BPC = B // NCORES
P = 128
G = 4                # partition groups stacked in score PSUM
HG = H * G           # 32
F32 = mybir.dt.float32
BF16 = mybir.dt.bfloat16
NP_BF16 = ml_dtypes.bfloat16

# per-batch plan: 9 full loads (S=16 subtiles = 2048 rows), one S=12 load,
# then a 32-row tail handled separately.
FULL_LOADS = 9
PART_S = 12          # subtiles in the partial load (1536 rows)
TAIL_N0 = FULL_LOADS * 2048 + PART_S * 128   # 19968
TAIL_ROWS = N - TAIL_N0                      # 32
NSETS = FULL_LOADS + 1                       # score psum sets per batch
ACT_COPY_FRAC = 0.44  # fraction of PSUM->SBUF copies done on ACT


def build_kernel(bpc=BPC, reps=1):
    nc = bacc.Bacc("TRN2", target_bir_lowering=False, debug=False)
    k_in = nc.declare_dram_parameter("k", [bpc, N, D], F32, isOutput=False)
    id_in = nc.declare_dram_parameter("idbf", [P, P], BF16, isOutput=False)
    w32_in = nc.declare_dram_parameter("w32", [P, 2 * G * HG], BF16, isOutput=False)
    qb32_in = nc.declare_dram_parameter("qb32", [HG, bpc], F32, isOutput=False)
    fold32_in = nc.declare_dram_parameter("fold32", [HG, HG], F32, isOutput=False)
    out = nc.declare_dram_parameter("out", [bpc, H, N], F32, isOutput=True)

    with ExitStack() as ctx:
        tc = ctx.enter_context(tile.TileContext(nc))
        consts = ctx.enter_context(tc.tile_pool(name="consts", bufs=1))
        loads = ctx.enter_context(tc.tile_pool(name="loads", bufs=16))
        kts = ctx.enter_context(tc.tile_pool(name="kts", bufs=4))
        probp = ctx.enter_context(tc.tile_pool(name="prob", bufs=2))
        small = ctx.enter_context(tc.tile_pool(name="small", bufs=2))
        psum_kt = ctx.enter_context(tc.tile_pool(name="psum_kt", bufs=3, space="PSUM"))
        psum_sc = ctx.enter_context(tc.tile_pool(name="psum_sc", bufs=2, space="PSUM"))

        id_bf = consts.tile([P, P], BF16)
        nc.sync.dma_start(out=id_bf[:, :], in_=id_in[:, :])
        wk32 = consts.tile([P, 2, G, HG], BF16)
        nc.sync.dma_start(
            out=wk32[:, :, :, :],
            in_=w32_in[:, :].rearrange("p (d g x) -> p d g x", d=2, g=G),
        )
        qb32 = consts.tile([HG, bpc], F32)
        nc.sync.dma_start(out=qb32[:, :], in_=qb32_in[:, :])
        fold32 = consts.tile([HG, HG], F32)
        nc.sync.dma_start(out=fold32[:, :], in_=fold32_in[:, :])

        act_period = max(2, round(1.0 / max(ACT_COPY_FRAC, 1e-6)))
        copy_idx = [0]

        def copy_out(dst, src):
            i = copy_idx[0]
            copy_idx[0] += 1
            if i % act_period == 0:
                nc.scalar.copy(out=dst, in_=src)
            else:
                nc.vector.tensor_copy(out=dst, in_=src)

        # --- main loop -----------------------------------------------------
        first_iter = [True]
        for i in [ib for _ in range(reps) for ib in range(bpc)]:
            last_batch = i == bpc - 1
            # prob32[8g+h, 512m+c] = attn[h, perm(n)] (pre-scale)
            prob32 = probp.tile([HG, 512 * NSETS], F32, tag="prob")
            probT = probp.tile([H, TAIL_ROWS], F32, tag="probT")
            sums = small.tile([HG, NSETS + 1], F32, tag="sums")
            nc.vector.memset(sums[:, :], 0)

            for m in range(NSETS):
                S = 16 if m < FULL_LOADS else PART_S
                n0 = m * 2048
                rows = S * P
                kb = loads.tile([P, 16, 2, P], BF16, tag="load")
                # p-major source view: partition p holds rows n0+S*p..+S-1
                src = k_in[i, n0:n0 + rows, :].rearrange("(p s) d -> p s d", p=P)
                if first_iter[0]:
                    first_iter[0] = False
                    nchunk = 4            # quarters: shorter ramp
                elif S == 16:
                    nchunk = 2            # halves everywhere else
                else:
                    nchunk = 3            # partial set: 3 chunks of 4
                step = S // nchunk
                for q in range(nchunk):
                    nc.gpsimd.dma_start(
                        out=kb[:, q * step:(q + 1) * step, :, :],
                        in_=src[:, q * step:(q + 1) * step, :],
                    )
                ng = S // 4  # 512-col groups in this set (4 or 3)
                scp = psum_sc.tile([HG, 512], F32, tag="sc")
                for g in range(ng):
                    kt0 = psum_kt.tile([P, 512], F32, tag="kt0")
                    kt1 = psum_kt.tile([P, 512], F32, tag="kt1")
                    for t in range(4):
                        s = g * 4 + t
                        nc.tensor.matmul(
                            kt0[:, t * P:(t + 1) * P],
                            kb[:, s, 0, :],
                            id_bf[:, :],
                            start=True,
                            stop=True,
                        )
                        nc.tensor.matmul(
                            kt1[:, t * P:(t + 1) * P],
                            kb[:, s, 1, :],
                            id_bf[:, :],
                            start=True,
                            stop=True,
                        )
                    k0 = kts.tile([P, 512], BF16, tag="k0")
                    k1 = kts.tile([P, 512], BF16, tag="k1")
                    copy_out(k0[:, :], kt0[:, :])
                    copy_out(k1[:, :], kt1[:, :])
                    nc.tensor.matmul(
                        scp[:, :], wk32[:, 0, g, :], k0[:, :],
                        start=(g == 0), stop=False,
                    )
                    nc.tensor.matmul(
                        scp[:, :], wk32[:, 1, g, :], k1[:, :],
                        start=False, stop=(g == ng - 1),
                    )
                hg = ng * H
                nc.scalar.activation(
                    out=prob32[:hg, 512 * m:512 * (m + 1)],
                    in_=scp[:hg, :],
                    func=mybir.ActivationFunctionType.Exp,
                    bias=qb32[:hg, i:i + 1],
                    scale=1.0,
                )
                nc.vector.reduce_sum(
                    out=sums[:hg, m:m + 1],
                    in_=prob32[:hg, 512 * m:512 * (m + 1)],
                    axis=mybir.AxisListType.X,
                )

            # ---- 32-row tail (natural n order) ----------------------------
            kbt = loads.tile([P, 16, 2, P], BF16, tag="load")
            nc.gpsimd.dma_start(
                out=kbt[:TAIL_ROWS, :1, :, :],
                in_=k_in[i, TAIL_N0:N, :].rearrange("(s p) d -> p s d", p=TAIL_ROWS),
            )
            ktt0 = psum_kt.tile([P, 512], F32, tag="kt0")
            ktt1 = psum_kt.tile([P, 512], F32, tag="kt1")
            nc.tensor.matmul(
                ktt0[:, :TAIL_ROWS], kbt[:TAIL_ROWS, 0, 0, :],
                id_bf[:TAIL_ROWS, :TAIL_ROWS], start=True, stop=True,
            )
            nc.tensor.matmul(
                ktt1[:, :TAIL_ROWS], kbt[:TAIL_ROWS, 0, 1, :],
                id_bf[:TAIL_ROWS, :TAIL_ROWS], start=True, stop=True,
            )
            kt_s = kts.tile([P, 512], BF16, tag="k0")
            copy_out(kt_s[:, :TAIL_ROWS], ktt0[:, :TAIL_ROWS])
            copy_out(kt_s[:, 64:64 + TAIL_ROWS], ktt1[:, :TAIL_ROWS])
            sct = psum_sc.tile([HG, 512], F32, tag="sc")
            nc.tensor.matmul(
                sct[:H, :TAIL_ROWS], wk32[:, 0, 0, :H], kt_s[:, :TAIL_ROWS],
                start=True, stop=False,
            )
            nc.tensor.matmul(
                sct[:H, :TAIL_ROWS], wk32[:, 1, 0, :H],
                kt_s[:, 64:64 + TAIL_ROWS], start=False, stop=True,
            )
            nc.scalar.activation(
                out=probT[:, :],
                in_=sct[:H, :TAIL_ROWS],
                func=mybir.ActivationFunctionType.Exp,
                bias=qb32[:H, i:i + 1],
                scale=1.0,
                accum_out=sums[:H, NSETS:NSETS + 1],
            )

            # ---- totals, scale, store -------------------------------------
            srow = small.tile([HG, 1], F32, tag="srow")
            nc.vector.reduce_sum(out=srow[:, :], in_=sums[:, :], axis=mybir.AxisListType.X)
            totp = psum_sc.tile([HG, 512], F32, tag="sc")
            nc.tensor.matmul(totp[:, :1], fold32[:, :], srow[:, :])
            rec32 = small.tile([HG, 1], F32, tag="rec32")
            nc.vector.reciprocal(out=rec32[:, :], in_=totp[:, :1])

            # scale + store, interleaved per 2048-row set; muls round-robin
            # DVE/ACT/Pool, store DMAs round-robin so no single engine
            # serializes the tail (tensor/gpsimd only on the last batch,
            # when their main-loop work is done).
            mul_engs = [
                lambda seg, np_: nc.vector.tensor_scalar_mul(seg, seg, rec32[:np_, :]),
                lambda seg, np_: nc.scalar.mul(seg, seg, rec32[:np_, :1]),
            ]
            if last_batch:
                st_engs = [nc.sync, nc.gpsimd]
            else:
                st_engs = [nc.sync]
            pg = PART_S // 4  # 3 groups in the partial set
            full_cols = 512 * FULL_LOADS  # 4608
            for m in range(FULL_LOADS):
                seg = prob32[:, 512 * m:512 * (m + 1)]
                mul_engs[m % len(mul_engs)](seg, HG)
                st_engs[m % len(st_engs)].dma_start(
                    out=out[i, :, 2048 * m:2048 * (m + 1)].rearrange(
                        "h (g c) -> g h c", c=512
                    ),
                    in_=seg,
                )
            segp = prob32[:H * pg, full_cols:full_cols + 512]
            mul_engs[FULL_LOADS % len(mul_engs)](segp, H * pg)
            st_engs[FULL_LOADS % len(st_engs)].dma_start(
                out=out[i, :, 2048 * FULL_LOADS:TAIL_N0].rearrange(
                    "h (g c) -> g h c", c=512
                ),
                in_=segp,
            )
            nc.scalar.mul(probT[:, :], probT[:, :], rec32[:H, :1])
            st_engs[(FULL_LOADS + 1) % len(st_engs)].dma_start(
                out=out[i, :, TAIL_N0:], in_=probT[:, :]
            )

    nc.compile()
    return nc


_NC_CACHE = {}


def _get_nc():
    if "nc" not in _NC_CACHE:
        _NC_CACHE["nc"] = build_kernel()
    return _NC_CACHE["nc"]


def make_in_maps(query, key, W, b):
    """Host-side precompute of the tiny operands + per-core input maps."""
    query = np.asarray(query, np.float32).reshape(B, D)
    key = np.ascontiguousarray(np.asarray(key, np.float32))
    W = np.asarray(W, np.float32)
    b = np.asarray(b, np.float32)

    Wq, Wk = W[:, :D], W[:, D:]                      # [H, D] each
    qb_all = query @ Wq.T + b[None, :]               # [B, H]
    WkT = Wk.T.astype(NP_BF16)                       # [D, H] bf16
    # wk32[p, dhalf, g, 8g+h] = WkT[dhalf*128 + p, h]
    wk32 = np.zeros((P, 2, G, HG), NP_BF16)
    for g in range(G):
        wk32[:, 0, g, g * H:(g + 1) * H] = WkT[:P]
        wk32[:, 1, g, g * H:(g + 1) * H] = WkT[P:]
    idbf = np.eye(P, dtype=NP_BF16)
    fold32 = np.tile(np.eye(H, dtype=np.float32), (G, G))       # [HG, HG]

    in_maps = []
    for c in range(NCORES):
        s = slice(BPC * c, BPC * (c + 1))
        qb32 = np.tile(np.ascontiguousarray(qb_all[s].T), (G, 1))  # [HG, bpc]
        in_maps.append(
            {
                "k": key[s],
                "idbf": idbf,
                "w32": np.ascontiguousarray(wk32.reshape(P, 2 * G * HG)),
                "qb32": np.ascontiguousarray(qb32),
                "fold32": fold32,
            }
        )
    return in_maps


def unpermute(raw):
    """Undo the p-major within-set n-permutation (device col -> true n).

    Full sets: dev col 2048m+512g+128j+p  holds true n 2048m+16p+4g+j.
    Partial:   dev col 18432+512g+128j+p  holds true n 18432+12p+4g+j.
    Tail (32): already in true order.
    """
    nb, nh = raw.shape[0], raw.shape[1]
    out = np.empty_like(raw)
    blk = raw[:, :, : FULL_LOADS * 2048].reshape(nb, nh, FULL_LOADS, G, 4, P)
    out[:, :, : FULL_LOADS * 2048] = blk.transpose(0, 1, 2, 5, 3, 4).reshape(
        nb, nh, FULL_LOADS * 2048
    )
    pb = raw[:, :, FULL_LOADS * 2048:TAIL_N0].reshape(nb, nh, PART_S // 4, 4, P)
    out[:, :, FULL_LOADS * 2048:TAIL_N0] = pb.transpose(0, 1, 4, 2, 3).reshape(
        nb, nh, PART_S * P
    )
    out[:, :, TAIL_N0:] = raw[:, :, TAIL_N0:]
    return out


def kernel(query, key, W, b):
    from concourse.bass_utils import run_bass_kernel_spmd

    nc = _get_nc()
    in_maps = make_in_maps(query, key, W, b)
    res = run_bass_kernel_spmd(nc, in_maps, list(range(NCORES))).results
    raw = np.concatenate([res[c]["out"] for c in range(NCORES)], axis=0)
    return unpermute(raw)
